# revision 2
# baseline (speedup 1.0000x reference)
"""Trainium2 Bass kernel v4: 9-pattern masked depthwise 3x3 conv, 2 branches.

Full problem: xh, xl [4, 16, 512, 512] fp32; wh, wl, mh, ml [9, 16, 3, 3].
out = stack([conv9(xh, wh*mh), conv9(xl, wl*ml)])  -> [2, 9, 4, 16, 510, 510]
with clamp(-128, 127) and round-half-even applied elementwise.

Sharding: pure data parallel over (branch, batch) = 8 slices, one per core.

v4 strategy (single-pass mod-4 class matmuls, fp16):
  - Output columns split by j mod 4 (s_out in 0..3).  Input columns are
    gathered into 6 classes cl: x[., 4u+cl] for cl in 0..5 (cl 4,5 are
    mod-4 parities 0,1 shifted one u-block).  The 3 taps dj in 0..2 of
    output class s land in classes s..s+2 at the SAME u, so ONE matmul
    per (channel, s_out) covers all 9 taps: K = 6cl x 16kk = 96 rows
    (lhsT zero outside classes s..s+2; matmul base partition must be
    0/32/64 so we contract the whole 96 and let zeros mask), M = 9
    patterns x 14 rows = 126, free = nw windows x 128 u.  fp16 matmuls
    run 1 cycle/row at any free size -> half the PE time of a 2-pass
    f32r scheme, and fp16 halves input DMA bytes.
  - Windows of 14 output rows (16 input rows), stride 14; 37 windows
    (window 36 at base 496 re-covers rows 496..511 -> no row padding).
    Input tiles hold window PAIRS [96, 16c, 2w, 128u] so HBM descriptors
    are 512B+ (full 360GB/s) and matmul free = 256.  Input = 6/4 class
    dup x 16/14 overlap x 2B = 14.2MB/core.
  - PSUM tile per (window-pair, channel) [126, 4s, 2w, 128u] = 2 banks,
    pool bufs=4: four slots in flight hide drain latency (4-bank/2-slot
    variants stall the PE: measured 265us vs 185us).
  - Drains f32 PSUM -> int8 SBUF (the convert hw rounds half-even and
    saturates, matching the reference's clamp+round): free 2048 per op,
    un-interleaving j = 4u+s via a strided out AP.  DVE/Act weighted
    interleave (DVE share .4655 ~ 1038/(1192+1038)) keeps both engines
    ~equally busy; this drain wall (~167us) is the kernel's roofline:
    every PSUM f32 element must cross DVE (0.96G/lane) or Act
    (1.2G/lane) once, Pool/GPSIMD cannot read PSUM.
  - Output int8 rides Pool SWDGE (Pool is otherwise idle) as one DMA
    per 4 channels issued right after their drains: 512B descriptors,
    38.2MB/core.  DMA_ENGINES total ~152us < drain wall.
  - fp16 quantization of x and w*m costs ~2e-3 rel l2 on the rounded
    outputs (tolerance 2e-2); exactness of int8 rounding verified on HW
    (only round-boundary +-1 flips vs the f32 reference).
"""

import numpy as np

import concourse.bacc as bacc
import concourse.mybir as mybir
from concourse.tile import TileContext
from concourse.bass_utils import run_bass_kernel_spmd

B, C, H, W = 4, 16, 512, 512
HO, WO = H - 2, W - 2
NK = 9
RW = 14           # output rows per window
KK = 16           # input rows per window
NWIN = 37         # windows: bases 0,14,...,490, then 496
NWP = 19          # window-pair tiles; tile 18 holds only window 36
U = 128           # u positions per class
M = NK * RW       # 126 matmul output rows: m = p*14 + r
NWARM = 16        # PE p-state warm-up matmuls
DVE_SHARE = 0.4655

F16 = mybir.dt.float16
F32 = mybir.dt.float32
I8 = mybir.dt.int8
ADD = mybir.AluOpType.add
Copy = mybir.ActivationFunctionType.Copy

_CACHE = {}


def _build_nc():
    nc = bacc.Bacc()
    xq = nc.declare_dram_parameter("xq", [NWP, 96, C, 2, U], F16, isOutput=False)
    lw = nc.declare_dram_parameter("lw", [96, C * 4, M], F16, isOutput=False)
    y = nc.declare_dram_parameter("y", [NWP, C, 2, M, 512], I8, isOutput=True)

    with TileContext(nc) as tc:
        with (
            tc.tile_pool(name="lwp", bufs=1) as lwp,
            tc.tile_pool(name="xp", bufs=3) as xp,
            tc.tile_pool(name="outp", bufs=2) as outp,
            tc.tile_pool(name="psp", bufs=4, space="PSUM") as psp,
        ):
            lwt = lwp.tile([96, C * 4, M], F16)
            nc.sync.dma_start(out=lwt[:, 0:16, :], in_=lw[:, 0:16, :])

            # PE p-state warm-up on a memset scratch (0.65->2.4GHz over 3us)
            wsrc = lwp.tile([128, 260], F16, name="warm_src")
            nc.gpsimd.memset(wsrc[:].bitcast(F32), 0.0)
            wps = psp.tile([M, 4, 2, U], F32, tag="pst", name="warm_ps")
            for _ in range(NWARM):
                nc.tensor.matmul(
                    wps[:, 0, :, :],
                    lhsT=wsrc[0:96, 0:M],
                    rhs=wsrc[0:96, 4 : 4 + 2 * U],
                    start=True,
                    stop=True,
                )

            ndr = 0

            def drain(dst, src):
                # weighted DVE/Act interleave balancing engine busy time
                nonlocal ndr
                ndr += 1
                if int(ndr * DVE_SHARE) != int((ndr - 1) * DVE_SHARE):
                    nc.vector.tensor_scalar(dst, src, 0.0, None, ADD)
                else:
                    nc.scalar.activation(dst, src, Copy)

            for t in range(NWP):
                nw = 1 if t == NWP - 1 else 2
                xt = xp.tile([96, C, 2, U], F16, tag="xt", name=f"xt{t}")
                if t == 0:
                    # interleave input c-chunks with lhsT chunks so the PE
                    # streams through c blocks without waiting on either
                    for ch in range(4):
                        nc.sync.dma_start(
                            out=xt[:, 4 * ch : 4 * (ch + 1), :, :],
                            in_=xq[t][:, 4 * ch : 4 * (ch + 1), :, :],
                        )
                        if ch >= 1:
                            nc.sync.dma_start(
                                out=lwt[:, 16 * ch : 16 * (ch + 1), :],
                                in_=lw[:, 16 * ch : 16 * (ch + 1), :],
                            )
                elif nw == 2:
                    nc.sync.dma_start(out=xt[:], in_=xq[t])
                else:
                    nc.sync.dma_start(out=xt[:, :, 0, :], in_=xq[t][:, :, 0, :])

                ot = outp.tile([M, C, 2, 512], I8, tag="ot", name=f"ot{t}")

                if nw == 1:
                    # final single-window tile: per-c PSUM is 1 bank, so
                    # pair channels to keep the 2-bank / 2048-free drains
                    for cp in range(8):
                        pst = psp.tile([M, 2, 4, 1, U], F32, tag="pst", name=f"ps{t}_{cp}")
                        for ci in range(2):
                            c = 2 * cp + ci
                            for s in range(4):
                                nc.tensor.matmul(
                                    pst[:, ci, s, 0, :],
                                    lhsT=lwt[:, c * 4 + s, :],
                                    rhs=xt[:, c, 0, :],
                                    start=True,
                                    stop=True,
                                )
                        drain(
                            ot[:, 2 * cp : 2 * cp + 2, 0, :].rearrange(
                                "m ci (u s) -> m ci s u", s=4
                            ),
                            pst[:, :, :, 0, :],
                        )
                        if cp % 2 == 1:
                            c0 = 2 * cp - 2
                            nc.gpsimd.dma_start(
                                out=y[t, c0 : c0 + 4, 0:1].rearrange("c w m j -> m c w j"),
                                in_=ot[:, c0 : c0 + 4, 0:1, :],
                            )
                    continue

                for c in range(C):
                    pst = psp.tile([M, 4, 2, U], F32, tag="pst", name=f"ps{t}_{c}")
                    for s in range(4):
                        nc.tensor.matmul(
                            pst[:, s, 0:nw, :],
                            lhsT=lwt[:, c * 4 + s, :],
                            rhs=xt[:, c, 0:nw, :],
                            start=True,
                            stop=True,
                        )
                    drain(
                        ot[:, c, 0:nw, :].rearrange("m w (u s) -> m s w u", s=4),
                        pst[:, :, 0:nw, :],
                    )
                    if c % 4 == 3:
                        # output per 4 channels right after their drains:
                        # Pool SWDGE (idle engine), 512B descriptors
                        nc.gpsimd.dma_start(
                            out=y[t, c - 3 : c + 1, 0:nw].rearrange("c w m j -> m c w j"),
                            in_=ot[:, c - 3 : c + 1, 0:nw, :],
                        )
    return nc


def _get_nc():
    if "nc" not in _CACHE:
        nc = _build_nc()
        nc.finalize()
        _CACHE["nc"] = nc
    return _CACHE["nc"]


def _host_xq(xb):
    """xb [C, H, W] f32 -> xq [NWP, 96, C, 2, U] fp16."""
    xf = xb.astype(np.float16)
    xp = np.concatenate([xf, xf[:, :, -1:], xf[:, :, -1:]], axis=2)  # [C,512,514]
    wbase = np.minimum(14 * np.arange(NWIN), H - KK)  # 0,14,...,490,496
    rows = wbase[:, None] + np.arange(KK)[None, :]  # [37, 16]
    cols = 4 * np.arange(U)[None, :] + np.arange(6)[:, None]  # [6, 128]
    g = xp[:, rows]          # [C, 37, 16, 514]
    g = g[:, :, :, cols]     # [C, 37, 16, 6, 128]
    # -> [37w, 6cl, 16kk, C, 128u]
    g = g.transpose(1, 3, 2, 0, 4)
    out = np.zeros((NWP, 6, KK, C, 2, U), np.float16)
    out[:, :, :, :, 0] = g[0::2]
    out[: NWP - 1, :, :, :, 1] = g[1::2]
    return np.ascontiguousarray(out.reshape(NWP, 96, C, 2, U))


def _host_lw(wm):
    """wm = (w*m) [9, C, 3, 3] f32 -> lw [96, C*4, M] fp16."""
    wm16 = wm.astype(np.float32).astype(np.float16)
    lw = np.zeros((96, C * 4, M), np.float16)
    cc = np.arange(C)
    pp = np.arange(NK)
    for s in range(4):
        for dj in range(3):
            for di in range(3):
                for r in range(RW):
                    lw[
                        (s + dj) * 16 + r + di,
                        cc[:, None] * 4 + s,
                        pp[None, :] * 14 + r,
                    ] = wm16[:, :, di, dj].T
    return lw


def _in_maps(xh, xl, wh, wl, mh, ml):
    xh = np.asarray(xh, np.float32)
    xl = np.asarray(xl, np.float32)
    wmh = np.asarray(wh, np.float32) * np.asarray(mh, np.float32)
    wml = np.asarray(wl, np.float32) * np.asarray(ml, np.float32)
    maps = []
    for x_all, wm in [(xh, wmh), (xl, wml)]:
        lw_b = _host_lw(wm)
        for b in range(B):
            maps.append({"xq": _host_xq(x_all[b]), "lw": lw_b})
    return maps


def _reconstruct(yD):
    """yD [NWP, C, 2, M, 512] i8 -> [9, C, HO, WO] f32."""
    out = np.empty((NK, C, HO, WO), dtype=np.float32)
    wbase = np.minimum(14 * np.arange(NWIN), H - KK)
    for w in range(NWIN):
        blk = yD[w // 2, :, w % 2]  # [C, 126, 512]
        blk = blk.reshape(C, NK, RW, 512).transpose(1, 0, 2, 3)
        out[:, :, wbase[w] : wbase[w] + RW, :] = blk[:, :, :, :WO].astype(np.float32)
    return out


def kernel(xh, xl, wh, wl, mh, ml, h=0):
    nc = _get_nc()
    in_maps = _in_maps(xh, xl, wh, wl, mh, ml)
    res = run_bass_kernel_spmd(nc, in_maps, list(range(8)))
    out = np.empty((2, NK, B, C, HO, WO), dtype=np.float32)
    for core, rmap in enumerate(res.results):
        br, b = divmod(core, B)
        out[br, :, b] = _reconstruct(np.asarray(rmap["y"]))
    return out


if __name__ == "__main__":
    rng = np.random.RandomState(0)
    ins = {
        "xh": rng.randn(B, C, H, W).astype(np.float32) * 20,
        "xl": rng.randn(B, C, H, W).astype(np.float32) * 20,
        "wh": rng.randn(NK, C, 3, 3).astype(np.float32),
        "wl": rng.randn(NK, C, 3, 3).astype(np.float32),
        "mh": np.round(rng.rand(NK, C, 3, 3)).astype(np.float32),
        "ml": np.round(rng.rand(NK, C, 3, 3)).astype(np.float32),
        "h": 0,
    }
    out = kernel(**ins)
    print("kernel out:", out.shape, out.dtype, out.min(), out.max())


# revision 4
# speedup vs baseline: 1.0143x; 1.0143x over previous
"""Trainium2 Bass kernel v4: 9-pattern masked depthwise 3x3 conv, 2 branches.

Full problem: xh, xl [4, 16, 512, 512] fp32; wh, wl, mh, ml [9, 16, 3, 3].
out = stack([conv9(xh, wh*mh), conv9(xl, wl*ml)])  -> [2, 9, 4, 16, 510, 510]
with clamp(-128, 127) and round-half-even applied elementwise.

Sharding: pure data parallel over (branch, batch) = 8 slices, one per core.

v4 strategy (single-pass mod-4 class matmuls, fp16):
  - Output columns split by j mod 4 (s_out in 0..3).  Input columns are
    gathered into 6 classes cl: x[., 4u+cl] for cl in 0..5 (cl 4,5 are
    mod-4 parities 0,1 shifted one u-block).  The 3 taps dj in 0..2 of
    output class s land in classes s..s+2 at the SAME u, so ONE matmul
    per (channel, s_out) covers all 9 taps: K = 6cl x 16kk = 96 rows
    (lhsT zero outside classes s..s+2; matmul base partition must be
    0/32/64 so we contract the whole 96 and let zeros mask), M = 9
    patterns x 14 rows = 126, free = nw windows x 128 u.  fp16 matmuls
    run 1 cycle/row at any free size -> half the PE time of a 2-pass
    f32r scheme, and fp16 halves input DMA bytes.
  - Windows of 14 output rows (16 input rows), stride 14; 37 windows
    (window 36 at base 496 re-covers rows 496..511 -> no row padding).
    Input tiles hold window PAIRS [96, 16c, 2w, 128u] so HBM descriptors
    are 512B+ (full 360GB/s) and matmul free = 256.  Input = 6/4 class
    dup x 16/14 overlap x 2B = 14.2MB/core.
  - PSUM tile per (window-pair, channel) [126, 4s, 2w, 128u] = 2 banks,
    pool bufs=4: four slots in flight hide drain latency (4-bank/2-slot
    variants stall the PE: measured 265us vs 185us).
  - Drains f32 PSUM -> int8 SBUF (the convert hw rounds half-even and
    saturates, matching the reference's clamp+round): free 2048 per op,
    un-interleaving j = 4u+s via a strided out AP.  DVE/Act weighted
    interleave (DVE share .4655 ~ 1038/(1192+1038)) keeps both engines
    ~equally busy; this drain wall (~167us) is the kernel's roofline:
    every PSUM f32 element must cross DVE (0.96G/lane) or Act
    (1.2G/lane) once, Pool/GPSIMD cannot read PSUM.
  - Output int8 rides Pool SWDGE (Pool is otherwise idle) as one DMA
    per 4 channels issued right after their drains: 512B descriptors,
    38.2MB/core.  DMA_ENGINES total ~152us < drain wall.
  - fp16 quantization of x and w*m costs ~2e-3 rel l2 on the rounded
    outputs (tolerance 2e-2); exactness of int8 rounding verified on HW
    (only round-boundary +-1 flips vs the f32 reference).
"""

import numpy as np

import concourse.bacc as bacc
import concourse.mybir as mybir
from concourse.tile import TileContext
from concourse.bass_utils import run_bass_kernel_spmd

B, C, H, W = 4, 16, 512, 512
HO, WO = H - 2, W - 2
NK = 9
RW = 14           # output rows per window
KK = 16           # input rows per window
NWIN = 37         # windows: bases 0,14,...,490, then 496
NWP = 19          # window-pair tiles; tile 18 holds only window 36
U = 128           # u positions per class
M = NK * RW       # 126 matmul output rows: m = p*14 + r
NWARM = 8         # PE p-state warm-up matmuls
DVE_SHARE = 0.4655
DRAIN_PHASE = 21

F16 = mybir.dt.float16
F32 = mybir.dt.float32
I8 = mybir.dt.int8
ADD = mybir.AluOpType.add
Copy = mybir.ActivationFunctionType.Copy

_CACHE = {}


def _build_nc():
    nc = bacc.Bacc()
    xq = nc.declare_dram_parameter("xq", [NWP, 96, C, 2, U], F16, isOutput=False)
    lw = nc.declare_dram_parameter("lw", [96, C * 4, M], F16, isOutput=False)
    y = nc.declare_dram_parameter("y", [NWP, C, 2, M, 512], I8, isOutput=True)

    with TileContext(nc) as tc:
        with (
            tc.tile_pool(name="lwp", bufs=1) as lwp,
            tc.tile_pool(name="xp", bufs=3) as xp,
            tc.tile_pool(name="outp", bufs=2) as outp,
            tc.tile_pool(name="psp", bufs=4, space="PSUM") as psp,
        ):
            lwt = lwp.tile([96, C * 4, M], F16)
            nc.sync.dma_start(out=lwt[:, 0:16, :], in_=lw[:, 0:16, :])

            # PE p-state warm-up on a memset scratch (0.65->2.4GHz over 3us)
            wsrc = lwp.tile([128, 260], F16, name="warm_src")
            nc.gpsimd.memset(wsrc[:].bitcast(F32), 0.0)
            wps = psp.tile([M, 4, 2, U], F32, tag="pst", name="warm_ps")
            for _ in range(NWARM):
                nc.tensor.matmul(
                    wps[:, 0, :, :],
                    lhsT=wsrc[0:96, 0:M],
                    rhs=wsrc[0:96, 4 : 4 + 2 * U],
                    start=True,
                    stop=True,
                )

            ndr = DRAIN_PHASE

            def drain(dst, src):
                # weighted DVE/Act interleave balancing engine busy time
                nonlocal ndr
                ndr += 1
                if int(ndr * DVE_SHARE) != int((ndr - 1) * DVE_SHARE):
                    nc.vector.tensor_scalar(dst, src, 0.0, None, ADD)
                else:
                    nc.scalar.activation(dst, src, Copy)

            for t in range(NWP):
                nw = 1 if t == NWP - 1 else 2
                xt = xp.tile([96, C, 2, U], F16, tag="xt", name=f"xt{t}")
                if t == 0:
                    # interleave input c-chunks with lhsT chunks so the PE
                    # streams through c blocks without waiting on either
                    for ch in range(4):
                        nc.sync.dma_start(
                            out=xt[:, 4 * ch : 4 * (ch + 1), :, :],
                            in_=xq[t][:, 4 * ch : 4 * (ch + 1), :, :],
                        )
                        if ch >= 1:
                            nc.sync.dma_start(
                                out=lwt[:, 16 * ch : 16 * (ch + 1), :],
                                in_=lw[:, 16 * ch : 16 * (ch + 1), :],
                            )
                elif nw == 2:
                    nc.sync.dma_start(out=xt[:], in_=xq[t])
                else:
                    nc.sync.dma_start(out=xt[:, :, 0, :], in_=xq[t][:, :, 0, :])

                ot = outp.tile([M, C, 2, 512], I8, tag="ot", name=f"ot{t}")

                if nw == 1:
                    # final single-window tile: per-c PSUM is 1 bank, so
                    # pair channels to keep the 2-bank / 2048-free drains
                    for cp in range(8):
                        pst = psp.tile([M, 2, 4, 1, U], F32, tag="pst", name=f"ps{t}_{cp}")
                        for ci in range(2):
                            c = 2 * cp + ci
                            for s in range(4):
                                nc.tensor.matmul(
                                    pst[:, ci, s, 0, :],
                                    lhsT=lwt[:, c * 4 + s, :],
                                    rhs=xt[:, c, 0, :],
                                    start=True,
                                    stop=True,
                                )
                        drain(
                            ot[:, 2 * cp : 2 * cp + 2, 0, :].rearrange(
                                "m ci (u s) -> m ci s u", s=4
                            ),
                            pst[:, :, :, 0, :],
                        )
                        if cp % 2 == 1:
                            c0 = 2 * cp - 2
                            nc.gpsimd.dma_start(
                                out=y[t, c0 : c0 + 4, 0:1].rearrange("c w m j -> m c w j"),
                                in_=ot[:, c0 : c0 + 4, 0:1, :],
                            )
                    continue

                for c in range(C):
                    pst = psp.tile([M, 4, 2, U], F32, tag="pst", name=f"ps{t}_{c}")
                    for s in range(4):
                        nc.tensor.matmul(
                            pst[:, s, 0:nw, :],
                            lhsT=lwt[:, c * 4 + s, :],
                            rhs=xt[:, c, 0:nw, :],
                            start=True,
                            stop=True,
                        )
                    drain(
                        ot[:, c, 0:nw, :].rearrange("m w (u s) -> m s w u", s=4),
                        pst[:, :, 0:nw, :],
                    )
                    if c % 4 == 3:
                        # output per 4 channels right after their drains:
                        # Pool SWDGE (idle engine), 512B descriptors
                        nc.gpsimd.dma_start(
                            out=y[t, c - 3 : c + 1, 0:nw].rearrange("c w m j -> m c w j"),
                            in_=ot[:, c - 3 : c + 1, 0:nw, :],
                        )
    return nc


def _get_nc():
    if "nc" not in _CACHE:
        nc = _build_nc()
        nc.finalize()
        _CACHE["nc"] = nc
    return _CACHE["nc"]


def _host_xq(xb):
    """xb [C, H, W] f32 -> xq [NWP, 96, C, 2, U] fp16."""
    xf = xb.astype(np.float16)
    xp = np.concatenate([xf, xf[:, :, -1:], xf[:, :, -1:]], axis=2)  # [C,512,514]
    wbase = np.minimum(14 * np.arange(NWIN), H - KK)  # 0,14,...,490,496
    rows = wbase[:, None] + np.arange(KK)[None, :]  # [37, 16]
    cols = 4 * np.arange(U)[None, :] + np.arange(6)[:, None]  # [6, 128]
    g = xp[:, rows]          # [C, 37, 16, 514]
    g = g[:, :, :, cols]     # [C, 37, 16, 6, 128]
    # -> [37w, 6cl, 16kk, C, 128u]
    g = g.transpose(1, 3, 2, 0, 4)
    out = np.zeros((NWP, 6, KK, C, 2, U), np.float16)
    out[:, :, :, :, 0] = g[0::2]
    out[: NWP - 1, :, :, :, 1] = g[1::2]
    return np.ascontiguousarray(out.reshape(NWP, 96, C, 2, U))


def _host_lw(wm):
    """wm = (w*m) [9, C, 3, 3] f32 -> lw [96, C*4, M] fp16."""
    wm16 = wm.astype(np.float32).astype(np.float16)
    lw = np.zeros((96, C * 4, M), np.float16)
    cc = np.arange(C)
    pp = np.arange(NK)
    for s in range(4):
        for dj in range(3):
            for di in range(3):
                for r in range(RW):
                    lw[
                        (s + dj) * 16 + r + di,
                        cc[:, None] * 4 + s,
                        pp[None, :] * 14 + r,
                    ] = wm16[:, :, di, dj].T
    return lw


def _in_maps(xh, xl, wh, wl, mh, ml):
    xh = np.asarray(xh, np.float32)
    xl = np.asarray(xl, np.float32)
    wmh = np.asarray(wh, np.float32) * np.asarray(mh, np.float32)
    wml = np.asarray(wl, np.float32) * np.asarray(ml, np.float32)
    maps = []
    for x_all, wm in [(xh, wmh), (xl, wml)]:
        lw_b = _host_lw(wm)
        for b in range(B):
            maps.append({"xq": _host_xq(x_all[b]), "lw": lw_b})
    return maps


def _reconstruct(yD):
    """yD [NWP, C, 2, M, 512] i8 -> [9, C, HO, WO] f32."""
    out = np.empty((NK, C, HO, WO), dtype=np.float32)
    wbase = np.minimum(14 * np.arange(NWIN), H - KK)
    for w in range(NWIN):
        blk = yD[w // 2, :, w % 2]  # [C, 126, 512]
        blk = blk.reshape(C, NK, RW, 512).transpose(1, 0, 2, 3)
        out[:, :, wbase[w] : wbase[w] + RW, :] = blk[:, :, :, :WO].astype(np.float32)
    return out


def kernel(xh, xl, wh, wl, mh, ml, h=0):
    nc = _get_nc()
    in_maps = _in_maps(xh, xl, wh, wl, mh, ml)
    res = run_bass_kernel_spmd(nc, in_maps, list(range(8)))
    out = np.empty((2, NK, B, C, HO, WO), dtype=np.float32)
    for core, rmap in enumerate(res.results):
        br, b = divmod(core, B)
        out[br, :, b] = _reconstruct(np.asarray(rmap["y"]))
    return out


if __name__ == "__main__":
    rng = np.random.RandomState(0)
    ins = {
        "xh": rng.randn(B, C, H, W).astype(np.float32) * 20,
        "xl": rng.randn(B, C, H, W).astype(np.float32) * 20,
        "wh": rng.randn(NK, C, 3, 3).astype(np.float32),
        "wl": rng.randn(NK, C, 3, 3).astype(np.float32),
        "mh": np.round(rng.rand(NK, C, 3, 3)).astype(np.float32),
        "ml": np.round(rng.rand(NK, C, 3, 3)).astype(np.float32),
        "h": 0,
    }
    out = kernel(**ins)
    print("kernel out:", out.shape, out.dtype, out.min(), out.max())
